# revision 21
# baseline (speedup 1.0000x reference)
"""Trainium2 Bass kernel for nn_LASLNNet (complex-valued 4D CNN).

Strategy (8 NeuronCores, SPMD single program):
  - core c handles (batch b = c//2, spatial half h = c%2) -> 4 x 2 split.
  - All complex convs are computed as real matmuls with doubled channels:
      [yr; yi] = [[Wr, Wi], [-Wi, Wr]]^T @ [xr; xi]
  - Every matmul keeps K uniform (the PE array reconfigures between
    different contraction sizes at a ~2.3x throughput penalty, so K=64
    slots are padded / packed rather than issued as narrow matmuls).
  - conv1 (k=3,s=2): im2col slabs on host (27 (j1,j2,j3) tap slabs,
    K=55 incl. a bias/ones row); j4 handled as 3 PSUM-accumulated
    K=64 matmuls with step-2 rhs reads. M=128: output channels are
    duplicated in the weight columns so PSUM partitions 64..127 carry a
    second copy used to build shifted x2 replicas without DMA.
  - x2 store: d4-padded flat grid [block(d1) 7, d2 9, d3 9, d4 10];
    x2t partitions 0..63 hold x2, partitions 64..127 hold x2 shifted +1
    (written by a second activation with dst offset -1, pad columns
    supplied by the initial memset). A second tile x2s810 holds x2 on
    partitions 0..63 (DVE block copies) and x2 shifted +810 = one d1
    block on partitions 64..127 (third activation per conv1 row).
  - conv2 (k=3,s=1,p=1): per (row, o2-group) PSUM accumulation of
    45 K=128 matmuls: 27 (j1,j2,j3) taps with j4 in {0,1} fused via the
    +1 replica; 9 (j2,j3) taps with j4=2, j1 in {0,1} fused via the
    +810 replica; 9 (j2,j3) taps with j4=2, j1=2 with zero upper-half
    weights. Edge taps restrict (o2,o3) ranges via strided APs; PSUM
    has_written semantics make partial-region accumulation correct
    (the first matmul of each group is the full-region interior tap).
  - conv3/4/5 (1x1): plain matmuls on a compact layout.
  - FC: on-chip mul+reduce against host-sliced fcw; final cross-half
    sum + fc bias on host (each core returns a [128,1] partial).
  - dtype: bf16 matmul operands, fp32 PSUM/copies.

Spatial split along first output spatial dim D1 (9 rows):
  half 0 -> conv2..4 rows 0..4, half 1 -> rows 4..8 (row 4 duplicated);
  conv5 rows {0,1,2} / {2,3,4} (row 2 duplicated, masked via zeroed fcw).
"""

import itertools

import numpy as np
import ml_dtypes

import concourse.bacc as bacc
import concourse.mybir as mybir
from concourse.tile import TileContext
from concourse.bass_utils import run_bass_kernel_spmd

F32 = mybir.dt.float32
BF16 = mybir.dt.bfloat16
BF = ml_dtypes.bfloat16

NB = 4            # batch
R1 = 7            # conv1 rows computed per core (incl. dummy edge rows)
R2 = 5            # conv2/3/4 rows per core
R5 = 3            # conv5 rows per core
D4P = 10          # d4-padded inner dim (9 valid + 1 zero)
BLK = 9 * 9 * D4P                # 810, one d1-block of x2
X2N = R1 * BLK                   # logical x2 elements per partition
S1R = 9 * 9 * 20                 # 1620, conv1 slab elements per row
S1N = R1 * S1R                   # 11340 conv1 slab elements per partition
N3 = R2 * 729                    # 3645 compact columns for conv3/4
N5 = R5 * 125                    # 375 conv5 output columns

_CACHE = {}


def _build_nc():
    nc = bacc.Bacc("TRN2", target_bir_lowering=False, debug=False)

    x1_d = nc.dram_tensor("x1", [64, S1N], BF16, kind="ExternalInput")
    w1_d = nc.dram_tensor("w1", [64, 3 * 128], BF16, kind="ExternalInput")
    w2a_d = nc.dram_tensor("w2a", [128, 27 * 128], BF16, kind="ExternalInput")
    w2c_d = nc.dram_tensor("w2c", [128, 9 * 128], BF16, kind="ExternalInput")
    w2d_d = nc.dram_tensor("w2d", [128, 9 * 128], BF16, kind="ExternalInput")
    b2_d = nc.dram_tensor("b2", [128, 1], F32, kind="ExternalInput")
    w3_d = nc.dram_tensor("w3", [128, 2 * 128], BF16, kind="ExternalInput")
    b3_d = nc.dram_tensor("b3", [128, 2], F32, kind="ExternalInput")
    w4_d = nc.dram_tensor("w4", [128, 4 * 128], BF16, kind="ExternalInput")
    b4_d = nc.dram_tensor("b4", [128, 2], F32, kind="ExternalInput")
    w5_d = nc.dram_tensor("w5", [128, 2 * 128], BF16, kind="ExternalInput")
    b5_d = nc.dram_tensor("b5", [128, 1], F32, kind="ExternalInput")
    fcw_d = nc.dram_tensor("fcw", [128, N5], F32, kind="ExternalInput")
    out_d = nc.dram_tensor("out", [128, 1], F32, kind="ExternalOutput")

    Relu = mybir.ActivationFunctionType.Relu

    with TileContext(nc) as tc:
        with tc.tile_pool(name="sb", bufs=1) as pool, \
             tc.tile_pool(name="ps", bufs=8, space="PSUM") as pp:
            x1t = pool.tile([64, S1N], BF16, tag="x1")
            w1t = pool.tile([64, 3 * 128], BF16, tag="w1")
            # x2 store: [1 lead margin][R1 blocks of BLK][1 tail margin]
            x2t = pool.tile([128, X2N + 92], BF16, tag="x2")
            x2s8 = pool.tile([128, X2N + 92], BF16, tag="x2s8")
            w2at = pool.tile([128, 27 * 128], BF16, tag="w2a")
            w2ct = pool.tile([128, 9 * 128], BF16, tag="w2c")
            w2dt = pool.tile([128, 9 * 128], BF16, tag="w2d")
            b2t = pool.tile([128, 1], F32, tag="b2")
            x3t = pool.tile([128, N3], BF16, tag="x3")
            w3t = pool.tile([128, 2 * 128], BF16, tag="w3")
            b3t = pool.tile([128, 2], F32, tag="b3")
            x4t = pool.tile([128, 2 * N3], BF16, tag="x4")
            x4bt = pool.tile([128, 2 * N3], BF16, tag="x4b")
            w4t = pool.tile([128, 4 * 128], BF16, tag="w4")
            b4t = pool.tile([128, 2], F32, tag="b4")
            w5t = pool.tile([128, 2 * 128], BF16, tag="w5")
            b5t = pool.tile([128, 1], F32, tag="b5")
            x5t = pool.tile([128, N5], F32, tag="x5")
            fcwt = pool.tile([128, N5], F32, tag="fcw")
            prodt = pool.tile([128, N5], F32, tag="prod")
            fct = pool.tile([128, 1], F32, tag="fc")

            # weights first (small w1 unblocks conv1), then x1 row chunks
            # so conv1 row r only waits for its own slab chunk.
            nc.sync.dma_start(w1t[:, :], w1_d[:, :])
            for (ra, rb) in ((0, 2), (2, 4), (4, 7)):
                nc.sync.dma_start(x1t[:, ra * S1R:rb * S1R],
                                  x1_d[:, ra * S1R:rb * S1R])
            nc.sync.dma_start(w2at[:, :], w2a_d[:, :])
            nc.sync.dma_start(w2ct[:, :], w2c_d[:, :])
            nc.sync.dma_start(w2dt[:, :], w2d_d[:, :])
            nc.sync.dma_start(b2t[:, :], b2_d[:, :])
            nc.sync.dma_start(w3t[:, :], w3_d[:, :])
            nc.sync.dma_start(b3t[:, :], b3_d[:, :])
            nc.sync.dma_start(w4t[:, :], w4_d[:, :])
            nc.sync.dma_start(b4t[:, :], b4_d[:, :])
            nc.sync.dma_start(w5t[:, :], w5_d[:, :])
            nc.sync.dma_start(b5t[:, :], b5_d[:, :])
            nc.sync.dma_start(fcwt[:, :], fcw_d[:, :])

            # ---------------- conv1 ----------------
            # slab view: [r(R1), o2(9), o3(9), d4(20)]
            s1v = x1t.rearrange("p (r a b c) -> p r a b c", r=R1, a=9, b=9, c=20)
            # x2 logical views. Lower (alloc offset 1): plain x2.
            x2v = x2t[:, 1:1 + X2N].rearrange(
                "p (r a b c) -> p r a b c", r=R1, a=9, b=9, c=D4P)
            # Upper of x2t (alloc offset 0): x2 shifted +1; the d4 pad
            # column of each cell is never written -> stays 0 from the
            # memset, which is exactly value x2[pad]=0 shifted into place.
            x2u = x2t[:, 0:X2N].rearrange(
                "p (r a b c) -> p r a b c", r=R1, a=9, b=9, c=D4P)
            # Upper of x2s8 (alloc offset 1): x2 shifted +810 (one block):
            # conv1 row r output is written at block slot r-1.
            x2s8u = x2s8[:, 1:1 + X2N].rearrange(
                "p (r a b c) -> p r a b c", r=R1, a=9, b=9, c=D4P)

            # Zero the grids (pad columns, margins, unwritten gaps). Must
            # cover full cells, not just the k=9 pads: a pads-only memset
            # is region-disjoint from the activations, so nothing would
            # order it before them, and k=8/k=9 share a 4-byte word ->
            # engine write race. Overlapping regions serialize via Tile
            # deps; gpsimd runs these during the x1 DMA, off the critical
            # path. x2s8's lower half needs no zeroing (DVE block copies
            # are its only writer and cover every read).
            nc.gpsimd.memset(x2t[0:64, 0:X2N + 1], 0)
            nc.gpsimd.memset(x2t[64:128, 0:X2N], 0)
            nc.gpsimd.memset(x2s8[64:128, 1:1 + X2N], 0)

            for r in range(R1):
                for (o2s, c2g) in ((0, 5), (5, 4)):
                    n = c2g * 81
                    ps1 = pp.tile([128, 512], F32, tag="ps")
                    ps1v = ps1[:, :n].rearrange("p (a b c) -> p a b c",
                                                a=c2g, b=9, c=9)
                    for j4 in range(3):
                        rhs = s1v[:, r, o2s:o2s + c2g, :, j4:j4 + 17:2]
                        nc.tensor.matmul(
                            ps1v[:, :, :, :],
                            w1t[:, j4 * 128:(j4 + 1) * 128],
                            rhs,
                            start=(j4 == 0), stop=(j4 == 2))
                    nc.scalar.activation(
                        x2v[0:64, r, o2s:o2s + c2g, :, 0:9],
                        ps1v[0:64, :, :, :],
                        Relu)
                    nc.vector.tensor_relu(
                        x2u[64:128, r, o2s:o2s + c2g, :, 0:9],
                        ps1v[64:128, :, :, :])
                    if r >= 1:
                        nc.scalar.activation(
                            x2s8u[64:128, r - 1, o2s:o2s + c2g, :, 0:9],
                            ps1v[64:128, :, :, :],
                            Relu)
                # lower half of x2s8: plain copy of the finished block
                # (same partitions -> DVE can do it; covers pad columns)
                if r < 5:
                    nc.vector.tensor_copy(
                        x2s8[0:64, 1 + r * BLK:1 + (r + 1) * BLK],
                        x2t[0:64, 1 + r * BLK:1 + (r + 1) * BLK])

            # ---------------- conv2 ----------------
            # taps ordered interior-first so the first matmul of each PSUM
            # group covers the full region (has_written correctness).
            taps = sorted(itertools.product(range(3), repeat=3),
                          key=lambda t: (t != (1, 1, 1)))
            taps9 = list(itertools.product(range(3), repeat=2))
            x3v = x3t.rearrange("p (r a b c) -> p r a b c", r=R2, a=9, b=9, c=9)

            def conv2_row(r):
                for (o2s, c2g) in ((0, 5), (5, 4)):
                    n = c2g * 81
                    ps2 = pp.tile([128, 512], F32, tag="ps")
                    ps2v = ps2[:, :n].rearrange("p (a b c) -> p a b c",
                                                a=c2g, b=9, c=9)

                    def region(j2, j3):
                        lo2 = max(o2s, 1 - j2)
                        hi2 = min(o2s + c2g, 10 - j2)
                        lo3 = max(0, 1 - j3)
                        hi3 = min(9, 10 - j3)
                        return lo2, hi2, lo3, hi3

                    # 27 (j1,j2,j3) taps, j4 in {0,1} via the +1 replica
                    for ti, (j1, j2, j3) in enumerate(taps):
                        lo2, hi2, lo3, hi3 = region(j2, j3)
                        c2 = hi2 - lo2
                        c3 = hi3 - lo3
                        out_ap = ps2v[:, lo2 - o2s:hi2 - o2s, lo3:hi3, :]
                        t27 = j1 * 9 + j2 * 3 + j3
                        # alloc base for (o2=lo2, o3=lo3, o4=0), j4=0 on the
                        # base partitions (the +1 alloc offset and the -1
                        # d4 pad shift cancel):
                        base0 = ((r + j1) * BLK + (lo2 + j2 - 1) * 90
                                 + (lo3 + j3 - 1) * D4P)
                        rhs0 = x2t[:, base0:base0 + c2 * 90].rearrange(
                            "p (a b c) -> p a b c", a=c2, b=9, c=D4P)[
                            :, :, 0:c3, 0:9]
                        nc.tensor.matmul(
                            out_ap,
                            w2at[:, t27 * 128:(t27 + 1) * 128],
                            rhs0,
                            start=(ti == 0), stop=False)
                    # 9 (j2,j3) taps, j4=2, j1 in {0,1} via the +810 replica
                    for t9, (j2, j3) in enumerate(taps9):
                        lo2, hi2, lo3, hi3 = region(j2, j3)
                        c2 = hi2 - lo2
                        c3 = hi3 - lo3
                        out_ap = ps2v[:, lo2 - o2s:hi2 - o2s, lo3:hi3, :]
                        base = (r * BLK + (lo2 + j2 - 1) * 90
                                + (lo3 + j3 - 1) * D4P + 2)
                        rhs = x2s8[:, base:base + c2 * 90].rearrange(
                            "p (a b c) -> p a b c", a=c2, b=9, c=D4P)[
                            :, :, 0:c3, 0:9]
                        nc.tensor.matmul(
                            out_ap,
                            w2ct[:, t9 * 128:(t9 + 1) * 128],
                            rhs,
                            start=False, stop=False)
                    # 9 (j2,j3) taps, j4=2, j1=2: zero upper-half weights
                    for t9, (j2, j3) in enumerate(taps9):
                        lo2, hi2, lo3, hi3 = region(j2, j3)
                        c2 = hi2 - lo2
                        c3 = hi3 - lo3
                        out_ap = ps2v[:, lo2 - o2s:hi2 - o2s, lo3:hi3, :]
                        base = ((r + 2) * BLK + (lo2 + j2 - 1) * 90
                                + (lo3 + j3 - 1) * D4P + 2)
                        rhs = x2t[:, base:base + c2 * 90].rearrange(
                            "p (a b c) -> p a b c", a=c2, b=9, c=D4P)[
                            :, :, 0:c3, 0:9]
                        nc.tensor.matmul(
                            out_ap,
                            w2dt[:, t9 * 128:(t9 + 1) * 128],
                            rhs,
                            start=False, stop=(t9 == 8))
                    nc.scalar.activation(
                        x3v[:, r, o2s:o2s + c2g, :, :],
                        ps2v[:, :, :, :],
                        Relu, bias=b2t[:, :])

            # ---------------- conv3/4/5 per-row emitters ----------------
            # Row-aligned chunks so a row's 1x1 convs can ride right
            # behind the producing activations (interleaved below).
            rchunks = ((0, 405), (405, 324))

            def conv3_row(r):
                for mh in range(2):
                    for (pos, sz) in rchunks:
                        p0 = r * 729 + pos
                        ps3 = pp.tile([128, 512], F32, tag="ps")
                        nc.tensor.matmul(
                            ps3[:, :sz],
                            w3t[:, mh * 128:(mh + 1) * 128],
                            x3t[:, p0:p0 + sz],
                            start=True, stop=True)
                        nc.scalar.activation(
                            x4t[:, mh * N3 + p0:mh * N3 + p0 + sz],
                            ps3[:, :sz],
                            Relu, bias=b3t[:, mh:mh + 1])

            def conv4_row(r):
                for mh in range(2):
                    for (pos, sz) in rchunks:
                        p0 = r * 729 + pos
                        ps4 = pp.tile([128, 512], F32, tag="ps")
                        nc.tensor.matmul(
                            ps4[:, :sz],
                            w4t[:, (mh * 2) * 128:(mh * 2 + 1) * 128],
                            x4t[:, p0:p0 + sz],
                            start=True, stop=False)
                        nc.tensor.matmul(
                            ps4[:, :sz],
                            w4t[:, (mh * 2 + 1) * 128:(mh * 2 + 2) * 128],
                            x4t[:, N3 + p0:N3 + p0 + sz],
                            start=False, stop=True)
                        nc.scalar.activation(
                            x4bt[:, mh * N3 + p0:mh * N3 + p0 + sz],
                            ps4[:, :sz],
                            Relu, bias=b4t[:, mh:mh + 1])

            # x4b view: [mb(2), r(R2), o2(9), o3(9), o4(9)]
            x4bv = x4bt.rearrange("p (m r a b c) -> p m r a b c",
                                  m=2, r=R2, a=9, b=9, c=9)

            def conv5_row(rr):
                ps5 = pp.tile([128, 512], F32, tag="ps")
                for mb in range(2):
                    rhs = x4bv[:, mb, 2 * rr, 0:9:2, 0:9:2, 0:9:2]
                    nc.tensor.matmul(
                        ps5[:, :125],
                        w5t[:, mb * 128:(mb + 1) * 128],
                        rhs,
                        start=(mb == 0), stop=(mb == 1))
                nc.scalar.activation(
                    x5t[:, rr * 125:(rr + 1) * 125],
                    ps5[:, :125],
                    Relu, bias=b5t[:, :])

            # Interleaved schedule: downstream layers lag far enough that
            # their input activations are already drained when the PE
            # reaches them, so the tensor engine never stalls.
            for r in range(R2):
                conv2_row(r)
                if r >= 1:
                    conv3_row(r - 1)
                if r >= 2:
                    conv4_row(r - 2)
                if r == 4:
                    conv5_row(0)
            conv3_row(4)
            conv4_row(3)
            conv5_row(1)
            conv4_row(4)
            conv5_row(2)

            # ---------------- FC partials ----------------
            nc.vector.tensor_mul(prodt[:, :], x5t[:, :], fcwt[:, :])
            nc.vector.reduce_sum(fct[:, :], prodt[:, :],
                                 axis=mybir.AxisListType.X)

            nc.sync.dma_start(out_d[:, :], fct[:, :])

    nc.compile()
    return nc


# ---------------- host-side data prep ----------------

def _cplx_block(wr_t, wi_t):
    """[32ci r; 32ci i] x [64co r | 64co i] real-matmul block."""
    f32 = np.float32
    B = np.zeros((64, 128), f32)
    B[0:32, 0:64] = wr_t
    B[0:32, 64:128] = wi_t
    B[32:64, 0:64] = -wi_t
    B[32:64, 64:128] = wr_t
    return B


def _prep_weights(inputs):
    f32 = np.float32
    w1r = np.asarray(inputs["w1r"], f32)[:, 0]   # [32, 3,3,3,3]
    w1i = np.asarray(inputs["w1i"], f32)[:, 0]
    # [t27, j4, co]
    w1r_t = w1r.transpose(1, 2, 3, 4, 0).reshape(27, 3, 32)
    w1i_t = w1i.transpose(1, 2, 3, 4, 0).reshape(27, 3, 32)
    # [64, 3*128]: per j4 block, M=128 with duplicated 64-wide halves so
    # PSUM partitions 64..127 carry a copy (used for shifted replicas).
    W1 = np.zeros((64, 3 * 128), f32)
    for j4 in range(3):
        blk = np.zeros((64, 64), f32)
        blk[0:27, 0:32] = w1r_t[:, j4]
        blk[0:27, 32:64] = w1i_t[:, j4]
        blk[27:54, 0:32] = -w1i_t[:, j4]
        blk[27:54, 32:64] = w1r_t[:, j4]
        if j4 == 0:
            blk[54, 0:32] = np.asarray(inputs["b1r"], f32)
            blk[54, 32:64] = np.asarray(inputs["b1i"], f32)
        W1[:, j4 * 128:j4 * 128 + 64] = blk
        W1[:, j4 * 128 + 64:(j4 + 1) * 128] = blk

    w2r = np.asarray(inputs["w2r"], f32)   # [64, 32, 3,3,3,3]
    w2i = np.asarray(inputs["w2i"], f32)
    # [j1, j2, j3, j4, ci, co]
    w2r_t = w2r.transpose(2, 3, 4, 5, 1, 0)
    w2i_t = w2i.transpose(2, 3, 4, 5, 1, 0)
    # w2a: 27 (j1,j2,j3) taps, rows 0:64 = j4=0, rows 64:128 = j4=1
    W2a = np.zeros((128, 27 * 128), f32)
    for t, (j1, j2, j3) in enumerate(itertools.product(range(3), repeat=3)):
        W2a[0:64, t * 128:(t + 1) * 128] = _cplx_block(
            w2r_t[j1, j2, j3, 0], w2i_t[j1, j2, j3, 0])
        W2a[64:128, t * 128:(t + 1) * 128] = _cplx_block(
            w2r_t[j1, j2, j3, 1], w2i_t[j1, j2, j3, 1])
    # w2c: 9 (j2,j3) taps at j4=2, rows 0:64 = j1=0, rows 64:128 = j1=1
    W2c = np.zeros((128, 9 * 128), f32)
    W2d = np.zeros((128, 9 * 128), f32)
    for t, (j2, j3) in enumerate(itertools.product(range(3), repeat=2)):
        W2c[0:64, t * 128:(t + 1) * 128] = _cplx_block(
            w2r_t[0, j2, j3, 2], w2i_t[0, j2, j3, 2])
        W2c[64:128, t * 128:(t + 1) * 128] = _cplx_block(
            w2r_t[1, j2, j3, 2], w2i_t[1, j2, j3, 2])
        W2d[0:64, t * 128:(t + 1) * 128] = _cplx_block(
            w2r_t[2, j2, j3, 2], w2i_t[2, j2, j3, 2])
    B2 = np.concatenate([np.asarray(inputs["b2r"], f32),
                         np.asarray(inputs["b2i"], f32)])[:, None]

    w3r = np.asarray(inputs["w3r"], f32).reshape(128, 64)
    w3i = np.asarray(inputs["w3i"], f32).reshape(128, 64)
    W3 = np.zeros((128, 2 * 128), f32)
    W3[0:64, 0:128] = w3r.T
    W3[64:128, 0:128] = -w3i.T
    W3[0:64, 128:256] = w3i.T
    W3[64:128, 128:256] = w3r.T
    B3 = np.stack([np.asarray(inputs["b3r"], f32),
                   np.asarray(inputs["b3i"], f32)], axis=1)

    w4r = np.asarray(inputs["w4r"], f32).reshape(128, 128)
    w4i = np.asarray(inputs["w4i"], f32).reshape(128, 128)
    W4 = np.zeros((128, 4 * 128), f32)
    W4[:, 0:128] = w4r.T
    W4[:, 128:256] = -w4i.T
    W4[:, 256:384] = w4i.T
    W4[:, 384:512] = w4r.T
    B4 = np.stack([np.asarray(inputs["b4r"], f32),
                   np.asarray(inputs["b4i"], f32)], axis=1)

    w5r = np.asarray(inputs["w5r"], f32).reshape(64, 128)
    w5i = np.asarray(inputs["w5i"], f32).reshape(64, 128)
    W5 = np.zeros((128, 2 * 128), f32)
    W5[:, 0:64] = w5r.T
    W5[:, 64:128] = w5i.T
    W5[:, 128:192] = -w5i.T
    W5[:, 192:256] = w5r.T
    B5 = np.concatenate([np.asarray(inputs["b5r"], f32),
                         np.asarray(inputs["b5i"], f32)])[:, None]

    return {
        "w1": W1.astype(BF), "w2a": W2a.astype(BF), "w2c": W2c.astype(BF),
        "w2d": W2d.astype(BF),
        "b2": B2, "w3": W3.astype(BF), "b3": B3, "w4": W4.astype(BF),
        "b4": B4, "w5": W5.astype(BF), "b5": B5,
    }


def _prep_x1(xr_b, xi_b, h):
    """Conv1 input slab for one (batch, half): [64, R1, 9, 9, 20] bf16."""
    S = np.zeros((64, R1, 9, 9, 20), np.float32)
    glo = max(0, 4 * h - 1)
    ghi = min(8, 4 * h + 5)
    rlo = glo - (4 * h - 1)
    rhi = ghi - (4 * h - 1) + 1
    for t, (j1, j2, j3) in enumerate(itertools.product(range(3), repeat=3)):
        subr = xr_b[j1:j1 + 17:2, j2:j2 + 17:2, j3:j3 + 17:2, :]
        subi = xi_b[j1:j1 + 17:2, j2:j2 + 17:2, j3:j3 + 17:2, :]
        S[t, rlo:rhi] = subr[glo:ghi + 1]
        S[27 + t, rlo:rhi] = subi[glo:ghi + 1]
    S[54, rlo:rhi] = 1.0
    return S.reshape(64, S1N).astype(BF)


def _prep_fcw(fcw, h):
    out = np.zeros((128, N5), np.float32)
    f = np.asarray(fcw, np.float32).reshape(-1)
    for rr in range(R5):
        g5 = rr + 2 * h
        if h == 1 and rr == 0:
            continue  # overlap row masked on half 1
        out[:, rr * 125:(rr + 1) * 125] = f[g5 * 125:(g5 + 1) * 125][None, :]
    return out


def _make_in_maps(inputs):
    wmaps = _prep_weights(inputs)
    xr = np.asarray(inputs["xr"], np.float32)
    xi = np.asarray(inputs["xi"], np.float32)
    fcw = inputs["fcw"]
    in_maps = []
    for core in range(8):
        b, h = core // 2, core % 2
        m = dict(wmaps)
        m["x1"] = _prep_x1(xr[b, 0], xi[b, 0], h)
        m["fcw"] = _prep_fcw(fcw, h)
        in_maps.append(m)
    return in_maps


def kernel(**inputs):
    if "nc" not in _CACHE:
        _CACHE["nc"] = _build_nc()
    nc = _CACHE["nc"]

    in_maps = _make_in_maps(inputs)

    res = run_bass_kernel_spmd(nc, in_maps, core_ids=list(range(8)))

    fcb = np.asarray(inputs["fcb"], np.float32)
    yr = np.zeros((NB, 64, 1), np.float32)
    yi = np.zeros((NB, 64, 1), np.float32)
    for b in range(NB):
        p0 = res.results[2 * b]["out"]
        p1 = res.results[2 * b + 1]["out"]
        s = p0 + p1
        yr[b] = s[0:64] + fcb[0]
        yi[b] = s[64:128]
    return np.stack([yr, yi]).astype(np.float32)


# revision 22
# speedup vs baseline: 1.0350x; 1.0350x over previous
"""Trainium2 Bass kernel for nn_LASLNNet (complex-valued 4D CNN).

Strategy (8 NeuronCores, SPMD single program):
  - core c handles (batch b = c//2, spatial half h = c%2) -> 4 x 2 split.
  - All complex convs are computed as real matmuls with doubled channels:
      [yr; yi] = [[Wr, Wi], [-Wi, Wr]]^T @ [xr; xi]
  - Every matmul keeps K uniform (the PE array reconfigures between
    different contraction sizes at a ~2.3x throughput penalty, so K=64
    slots are padded / packed rather than issued as narrow matmuls).
  - conv1 (k=3,s=2): im2col slabs on host (27 (j1,j2,j3) tap slabs,
    K=55 incl. a bias/ones row); j4 handled as 3 PSUM-accumulated
    K=64 matmuls with step-2 rhs reads. M=128: output channels are
    duplicated in the weight columns so PSUM partitions 64..127 carry a
    second copy used to build shifted x2 replicas without DMA.
  - x2 store: d4-padded flat grid [block(d1) 7, d2 9, d3 9, d4 10];
    x2t partitions 0..63 hold x2, partitions 64..127 hold x2 shifted +1
    (written by a second activation with dst offset -1, pad columns
    supplied by the initial memset). A second tile x2s810 holds x2 on
    partitions 0..63 (DVE block copies) and x2 shifted +810 = one d1
    block on partitions 64..127 (third activation per conv1 row).
  - conv2 (k=3,s=1,p=1): per (row, o2-group) PSUM accumulation of
    45 K=128 matmuls: 27 (j1,j2,j3) taps with j4 in {0,1} fused via the
    +1 replica; 9 (j2,j3) taps with j4=2, j1 in {0,1} fused via the
    +810 replica; 9 (j2,j3) taps with j4=2, j1=2 with zero upper-half
    weights. Edge taps restrict (o2,o3) ranges via strided APs; PSUM
    has_written semantics make partial-region accumulation correct
    (the first matmul of each group is the full-region interior tap).
  - conv3/4/5 (1x1): plain matmuls on a compact layout.
  - FC: on-chip mul+reduce against host-sliced fcw; final cross-half
    sum + fc bias on host (each core returns a [128,1] partial).
  - dtype: bf16 matmul operands, fp32 PSUM/copies.

Spatial split along first output spatial dim D1 (9 rows):
  half 0 -> conv2..4 rows 0..4, half 1 -> rows 4..8 (row 4 duplicated);
  conv5 rows {0,1,2} / {2,3,4} (row 2 duplicated, masked via zeroed fcw).
"""

import itertools

import numpy as np
import ml_dtypes

import concourse.bacc as bacc
import concourse.mybir as mybir
from concourse.tile import TileContext
from concourse.bass_utils import run_bass_kernel_spmd

F32 = mybir.dt.float32
BF16 = mybir.dt.bfloat16
BF = ml_dtypes.bfloat16

NB = 4            # batch
R1 = 7            # conv1 rows computed per core (incl. dummy edge rows)
R2 = 5            # conv2/3/4 rows per core
R5 = 3            # conv5 rows per core
D4P = 10          # d4-padded inner dim (9 valid + 1 zero)
BLK = 9 * 9 * D4P                # 810, one d1-block of x2
X2N = R1 * BLK                   # logical x2 elements per partition
S1R = 9 * 9 * 20                 # 1620, conv1 slab elements per row
S1N = R1 * S1R                   # 11340 conv1 slab elements per partition
N3 = R2 * 729                    # 3645 compact columns for conv3/4
N5 = R5 * 125                    # 375 conv5 output columns

_CACHE = {}


def _build_nc():
    nc = bacc.Bacc("TRN2", target_bir_lowering=False, debug=False)

    x1_d = nc.dram_tensor("x1", [64, S1N], BF16, kind="ExternalInput")
    w1_d = nc.dram_tensor("w1", [64, 3 * 128], BF16, kind="ExternalInput")
    w2a_d = nc.dram_tensor("w2a", [128, 27 * 128], BF16, kind="ExternalInput")
    w2c_d = nc.dram_tensor("w2c", [128, 9 * 128], BF16, kind="ExternalInput")
    w2d_d = nc.dram_tensor("w2d", [128, 9 * 128], BF16, kind="ExternalInput")
    b2_d = nc.dram_tensor("b2", [128, 1], F32, kind="ExternalInput")
    w3_d = nc.dram_tensor("w3", [128, 2 * 128], BF16, kind="ExternalInput")
    b3_d = nc.dram_tensor("b3", [128, 2], F32, kind="ExternalInput")
    w4_d = nc.dram_tensor("w4", [128, 4 * 128], BF16, kind="ExternalInput")
    b4_d = nc.dram_tensor("b4", [128, 2], F32, kind="ExternalInput")
    w5_d = nc.dram_tensor("w5", [128, 2 * 128], BF16, kind="ExternalInput")
    b5_d = nc.dram_tensor("b5", [128, 1], F32, kind="ExternalInput")
    fcw_d = nc.dram_tensor("fcw", [128, N5], F32, kind="ExternalInput")
    out_d = nc.dram_tensor("out", [128, 1], F32, kind="ExternalOutput")

    Relu = mybir.ActivationFunctionType.Relu

    with TileContext(nc) as tc:
        with tc.tile_pool(name="sb", bufs=1) as pool, \
             tc.tile_pool(name="ps", bufs=8, space="PSUM") as pp:
            x1t = pool.tile([64, S1N], BF16, tag="x1")
            w1t = pool.tile([64, 3 * 128], BF16, tag="w1")
            # x2 store: [1 lead margin][R1 blocks of BLK][1 tail margin]
            x2t = pool.tile([128, X2N + 92], BF16, tag="x2")
            x2s8 = pool.tile([128, X2N + 92], BF16, tag="x2s8")
            w2at = pool.tile([128, 27 * 128], BF16, tag="w2a")
            w2ct = pool.tile([128, 9 * 128], BF16, tag="w2c")
            w2dt = pool.tile([128, 9 * 128], BF16, tag="w2d")
            b2t = pool.tile([128, 1], F32, tag="b2")
            x3t = pool.tile([128, N3], BF16, tag="x3")
            w3t = pool.tile([128, 2 * 128], BF16, tag="w3")
            b3t = pool.tile([128, 2], F32, tag="b3")
            x4t = pool.tile([128, 2 * N3], BF16, tag="x4")
            x4bt = pool.tile([128, 2 * N3], BF16, tag="x4b")
            w4t = pool.tile([128, 4 * 128], BF16, tag="w4")
            b4t = pool.tile([128, 2], F32, tag="b4")
            w5t = pool.tile([128, 2 * 128], BF16, tag="w5")
            b5t = pool.tile([128, 1], F32, tag="b5")
            x5t = pool.tile([128, N5], F32, tag="x5")
            fcwt = pool.tile([128, N5], F32, tag="fcw")
            prodt = pool.tile([128, N5], F32, tag="prod")
            fct = pool.tile([128, 1], F32, tag="fc")

            # weights first (small w1 unblocks conv1), then x1 row chunks
            # so conv1 row r only waits for its own slab chunk.
            nc.sync.dma_start(w1t[:, :], w1_d[:, :])
            for (ra, rb) in ((0, 2), (2, 4), (4, 7)):
                nc.sync.dma_start(x1t[:, ra * S1R:rb * S1R],
                                  x1_d[:, ra * S1R:rb * S1R])
            nc.sync.dma_start(w2at[:, :], w2a_d[:, :])
            nc.sync.dma_start(w2ct[:, :], w2c_d[:, :])
            nc.sync.dma_start(w2dt[:, :], w2d_d[:, :])
            nc.sync.dma_start(b2t[:, :], b2_d[:, :])
            nc.sync.dma_start(w3t[:, :], w3_d[:, :])
            nc.sync.dma_start(b3t[:, :], b3_d[:, :])
            nc.sync.dma_start(w4t[:, :], w4_d[:, :])
            nc.sync.dma_start(b4t[:, :], b4_d[:, :])
            nc.sync.dma_start(w5t[:, :], w5_d[:, :])
            nc.sync.dma_start(b5t[:, :], b5_d[:, :])
            nc.sync.dma_start(fcwt[:, :], fcw_d[:, :])

            # ---------------- conv1 ----------------
            # slab view: [r(R1), o2(9), o3(9), d4(20)]
            s1v = x1t.rearrange("p (r a b c) -> p r a b c", r=R1, a=9, b=9, c=20)
            # x2 logical views. Lower (alloc offset 1): plain x2.
            x2v = x2t[:, 1:1 + X2N].rearrange(
                "p (r a b c) -> p r a b c", r=R1, a=9, b=9, c=D4P)
            # Upper of x2t (alloc offset 0): x2 shifted +1; the d4 pad
            # column of each cell is never written -> stays 0 from the
            # memset, which is exactly value x2[pad]=0 shifted into place.
            x2u = x2t[:, 0:X2N].rearrange(
                "p (r a b c) -> p r a b c", r=R1, a=9, b=9, c=D4P)
            # Upper of x2s8 (alloc offset 1): x2 shifted +810 (one block):
            # conv1 row r output is written at block slot r-1.
            x2s8u = x2s8[:, 1:1 + X2N].rearrange(
                "p (r a b c) -> p r a b c", r=R1, a=9, b=9, c=D4P)

            # Zero the grids (pad columns, margins, unwritten gaps). Must
            # cover full cells, not just the k=9 pads: a pads-only memset
            # is region-disjoint from the activations, so nothing would
            # order it before them, and k=8/k=9 share a 4-byte word ->
            # engine write race. Overlapping regions serialize via Tile
            # deps; gpsimd runs these during the x1 DMA, off the critical
            # path. x2s8's lower half needs no zeroing (DVE block copies
            # are its only writer and cover every read).
            nc.gpsimd.memset(x2t[0:64, 0:X2N + 1], 0)
            nc.vector.memset(x2t[64:128, 0:X2N], 0)
            nc.vector.memset(x2s8[64:128, 1:1 + X2N], 0)

            for r in range(R1):
                for (o2s, c2g) in ((0, 5), (5, 4)):
                    n = c2g * 81
                    ps1 = pp.tile([128, 512], F32, tag="ps")
                    ps1v = ps1[:, :n].rearrange("p (a b c) -> p a b c",
                                                a=c2g, b=9, c=9)
                    for j4 in range(3):
                        rhs = s1v[:, r, o2s:o2s + c2g, :, j4:j4 + 17:2]
                        nc.tensor.matmul(
                            ps1v[:, :, :, :],
                            w1t[:, j4 * 128:(j4 + 1) * 128],
                            rhs,
                            start=(j4 == 0), stop=(j4 == 2))
                    nc.scalar.activation(
                        x2v[0:64, r, o2s:o2s + c2g, :, 0:9],
                        ps1v[0:64, :, :, :],
                        Relu)
                    nc.vector.tensor_relu(
                        x2u[64:128, r, o2s:o2s + c2g, :, 0:9],
                        ps1v[64:128, :, :, :])
                    if r >= 1:
                        nc.scalar.activation(
                            x2s8u[64:128, r - 1, o2s:o2s + c2g, :, 0:9],
                            ps1v[64:128, :, :, :],
                            Relu)
                # lower half of x2s8: plain copy of the finished block
                # (same partitions -> DVE can do it; covers pad columns)
                if r < 5:
                    nc.vector.tensor_copy(
                        x2s8[0:64, 1 + r * BLK:1 + (r + 1) * BLK],
                        x2t[0:64, 1 + r * BLK:1 + (r + 1) * BLK])

            # ---------------- conv2 ----------------
            # taps ordered interior-first so the first matmul of each PSUM
            # group covers the full region (has_written correctness).
            taps = sorted(itertools.product(range(3), repeat=3),
                          key=lambda t: (t != (1, 1, 1)))
            taps9 = list(itertools.product(range(3), repeat=2))
            x3v = x3t.rearrange("p (r a b c) -> p r a b c", r=R2, a=9, b=9, c=9)

            def conv2_row(r):
                for (o2s, c2g) in ((0, 5), (5, 4)):
                    n = c2g * 81
                    ps2 = pp.tile([128, 512], F32, tag="ps")
                    ps2v = ps2[:, :n].rearrange("p (a b c) -> p a b c",
                                                a=c2g, b=9, c=9)

                    def region(j2, j3):
                        lo2 = max(o2s, 1 - j2)
                        hi2 = min(o2s + c2g, 10 - j2)
                        lo3 = max(0, 1 - j3)
                        hi3 = min(9, 10 - j3)
                        return lo2, hi2, lo3, hi3

                    # 27 (j1,j2,j3) taps, j4 in {0,1} via the +1 replica
                    for ti, (j1, j2, j3) in enumerate(taps):
                        lo2, hi2, lo3, hi3 = region(j2, j3)
                        c2 = hi2 - lo2
                        c3 = hi3 - lo3
                        out_ap = ps2v[:, lo2 - o2s:hi2 - o2s, lo3:hi3, :]
                        t27 = j1 * 9 + j2 * 3 + j3
                        # alloc base for (o2=lo2, o3=lo3, o4=0), j4=0 on the
                        # base partitions (the +1 alloc offset and the -1
                        # d4 pad shift cancel):
                        base0 = ((r + j1) * BLK + (lo2 + j2 - 1) * 90
                                 + (lo3 + j3 - 1) * D4P)
                        rhs0 = x2t[:, base0:base0 + c2 * 90].rearrange(
                            "p (a b c) -> p a b c", a=c2, b=9, c=D4P)[
                            :, :, 0:c3, 0:9]
                        nc.tensor.matmul(
                            out_ap,
                            w2at[:, t27 * 128:(t27 + 1) * 128],
                            rhs0,
                            start=(ti == 0), stop=False)
                    # 9 (j2,j3) taps, j4=2, j1 in {0,1} via the +810 replica
                    for t9, (j2, j3) in enumerate(taps9):
                        lo2, hi2, lo3, hi3 = region(j2, j3)
                        c2 = hi2 - lo2
                        c3 = hi3 - lo3
                        out_ap = ps2v[:, lo2 - o2s:hi2 - o2s, lo3:hi3, :]
                        base = (r * BLK + (lo2 + j2 - 1) * 90
                                + (lo3 + j3 - 1) * D4P + 2)
                        rhs = x2s8[:, base:base + c2 * 90].rearrange(
                            "p (a b c) -> p a b c", a=c2, b=9, c=D4P)[
                            :, :, 0:c3, 0:9]
                        nc.tensor.matmul(
                            out_ap,
                            w2ct[:, t9 * 128:(t9 + 1) * 128],
                            rhs,
                            start=False, stop=False)
                    # 9 (j2,j3) taps, j4=2, j1=2: zero upper-half weights
                    for t9, (j2, j3) in enumerate(taps9):
                        lo2, hi2, lo3, hi3 = region(j2, j3)
                        c2 = hi2 - lo2
                        c3 = hi3 - lo3
                        out_ap = ps2v[:, lo2 - o2s:hi2 - o2s, lo3:hi3, :]
                        base = ((r + 2) * BLK + (lo2 + j2 - 1) * 90
                                + (lo3 + j3 - 1) * D4P + 2)
                        rhs = x2t[:, base:base + c2 * 90].rearrange(
                            "p (a b c) -> p a b c", a=c2, b=9, c=D4P)[
                            :, :, 0:c3, 0:9]
                        nc.tensor.matmul(
                            out_ap,
                            w2dt[:, t9 * 128:(t9 + 1) * 128],
                            rhs,
                            start=False, stop=(t9 == 8))
                    nc.scalar.activation(
                        x3v[:, r, o2s:o2s + c2g, :, :],
                        ps2v[:, :, :, :],
                        Relu, bias=b2t[:, :])

            # ---------------- conv3/4/5 per-row emitters ----------------
            # Row-aligned chunks so a row's 1x1 convs can ride right
            # behind the producing activations (interleaved below).
            rchunks = ((0, 405), (405, 324))

            def conv3_row(r):
                for mh in range(2):
                    for (pos, sz) in rchunks:
                        p0 = r * 729 + pos
                        ps3 = pp.tile([128, 512], F32, tag="ps")
                        nc.tensor.matmul(
                            ps3[:, :sz],
                            w3t[:, mh * 128:(mh + 1) * 128],
                            x3t[:, p0:p0 + sz],
                            start=True, stop=True)
                        nc.scalar.activation(
                            x4t[:, mh * N3 + p0:mh * N3 + p0 + sz],
                            ps3[:, :sz],
                            Relu, bias=b3t[:, mh:mh + 1])

            def conv4_row(r):
                for mh in range(2):
                    for (pos, sz) in rchunks:
                        p0 = r * 729 + pos
                        ps4 = pp.tile([128, 512], F32, tag="ps")
                        nc.tensor.matmul(
                            ps4[:, :sz],
                            w4t[:, (mh * 2) * 128:(mh * 2 + 1) * 128],
                            x4t[:, p0:p0 + sz],
                            start=True, stop=False)
                        nc.tensor.matmul(
                            ps4[:, :sz],
                            w4t[:, (mh * 2 + 1) * 128:(mh * 2 + 2) * 128],
                            x4t[:, N3 + p0:N3 + p0 + sz],
                            start=False, stop=True)
                        nc.scalar.activation(
                            x4bt[:, mh * N3 + p0:mh * N3 + p0 + sz],
                            ps4[:, :sz],
                            Relu, bias=b4t[:, mh:mh + 1])

            # x4b view: [mb(2), r(R2), o2(9), o3(9), o4(9)]
            x4bv = x4bt.rearrange("p (m r a b c) -> p m r a b c",
                                  m=2, r=R2, a=9, b=9, c=9)

            def conv5_row(rr):
                ps5 = pp.tile([128, 512], F32, tag="ps")
                for mb in range(2):
                    rhs = x4bv[:, mb, 2 * rr, 0:9:2, 0:9:2, 0:9:2]
                    nc.tensor.matmul(
                        ps5[:, :125],
                        w5t[:, mb * 128:(mb + 1) * 128],
                        rhs,
                        start=(mb == 0), stop=(mb == 1))
                nc.scalar.activation(
                    x5t[:, rr * 125:(rr + 1) * 125],
                    ps5[:, :125],
                    Relu, bias=b5t[:, :])

            # Interleaved schedule: downstream layers lag far enough that
            # their input activations are already drained when the PE
            # reaches them, so the tensor engine never stalls.
            for r in range(R2):
                conv2_row(r)
                if r >= 1:
                    conv3_row(r - 1)
                if r >= 2:
                    conv4_row(r - 2)
                if r == 4:
                    conv5_row(0)
            conv3_row(4)
            conv4_row(3)
            conv5_row(1)
            conv4_row(4)
            conv5_row(2)

            # ---------------- FC partials ----------------
            nc.vector.tensor_mul(prodt[:, :], x5t[:, :], fcwt[:, :])
            nc.vector.reduce_sum(fct[:, :], prodt[:, :],
                                 axis=mybir.AxisListType.X)

            nc.sync.dma_start(out_d[:, :], fct[:, :])

    nc.compile()
    return nc


# ---------------- host-side data prep ----------------

def _cplx_block(wr_t, wi_t):
    """[32ci r; 32ci i] x [64co r | 64co i] real-matmul block."""
    f32 = np.float32
    B = np.zeros((64, 128), f32)
    B[0:32, 0:64] = wr_t
    B[0:32, 64:128] = wi_t
    B[32:64, 0:64] = -wi_t
    B[32:64, 64:128] = wr_t
    return B


def _prep_weights(inputs):
    f32 = np.float32
    w1r = np.asarray(inputs["w1r"], f32)[:, 0]   # [32, 3,3,3,3]
    w1i = np.asarray(inputs["w1i"], f32)[:, 0]
    # [t27, j4, co]
    w1r_t = w1r.transpose(1, 2, 3, 4, 0).reshape(27, 3, 32)
    w1i_t = w1i.transpose(1, 2, 3, 4, 0).reshape(27, 3, 32)
    # [64, 3*128]: per j4 block, M=128 with duplicated 64-wide halves so
    # PSUM partitions 64..127 carry a copy (used for shifted replicas).
    W1 = np.zeros((64, 3 * 128), f32)
    for j4 in range(3):
        blk = np.zeros((64, 64), f32)
        blk[0:27, 0:32] = w1r_t[:, j4]
        blk[0:27, 32:64] = w1i_t[:, j4]
        blk[27:54, 0:32] = -w1i_t[:, j4]
        blk[27:54, 32:64] = w1r_t[:, j4]
        if j4 == 0:
            blk[54, 0:32] = np.asarray(inputs["b1r"], f32)
            blk[54, 32:64] = np.asarray(inputs["b1i"], f32)
        W1[:, j4 * 128:j4 * 128 + 64] = blk
        W1[:, j4 * 128 + 64:(j4 + 1) * 128] = blk

    w2r = np.asarray(inputs["w2r"], f32)   # [64, 32, 3,3,3,3]
    w2i = np.asarray(inputs["w2i"], f32)
    # [j1, j2, j3, j4, ci, co]
    w2r_t = w2r.transpose(2, 3, 4, 5, 1, 0)
    w2i_t = w2i.transpose(2, 3, 4, 5, 1, 0)
    # w2a: 27 (j1,j2,j3) taps, rows 0:64 = j4=0, rows 64:128 = j4=1
    W2a = np.zeros((128, 27 * 128), f32)
    for t, (j1, j2, j3) in enumerate(itertools.product(range(3), repeat=3)):
        W2a[0:64, t * 128:(t + 1) * 128] = _cplx_block(
            w2r_t[j1, j2, j3, 0], w2i_t[j1, j2, j3, 0])
        W2a[64:128, t * 128:(t + 1) * 128] = _cplx_block(
            w2r_t[j1, j2, j3, 1], w2i_t[j1, j2, j3, 1])
    # w2c: 9 (j2,j3) taps at j4=2, rows 0:64 = j1=0, rows 64:128 = j1=1
    W2c = np.zeros((128, 9 * 128), f32)
    W2d = np.zeros((128, 9 * 128), f32)
    for t, (j2, j3) in enumerate(itertools.product(range(3), repeat=2)):
        W2c[0:64, t * 128:(t + 1) * 128] = _cplx_block(
            w2r_t[0, j2, j3, 2], w2i_t[0, j2, j3, 2])
        W2c[64:128, t * 128:(t + 1) * 128] = _cplx_block(
            w2r_t[1, j2, j3, 2], w2i_t[1, j2, j3, 2])
        W2d[0:64, t * 128:(t + 1) * 128] = _cplx_block(
            w2r_t[2, j2, j3, 2], w2i_t[2, j2, j3, 2])
    B2 = np.concatenate([np.asarray(inputs["b2r"], f32),
                         np.asarray(inputs["b2i"], f32)])[:, None]

    w3r = np.asarray(inputs["w3r"], f32).reshape(128, 64)
    w3i = np.asarray(inputs["w3i"], f32).reshape(128, 64)
    W3 = np.zeros((128, 2 * 128), f32)
    W3[0:64, 0:128] = w3r.T
    W3[64:128, 0:128] = -w3i.T
    W3[0:64, 128:256] = w3i.T
    W3[64:128, 128:256] = w3r.T
    B3 = np.stack([np.asarray(inputs["b3r"], f32),
                   np.asarray(inputs["b3i"], f32)], axis=1)

    w4r = np.asarray(inputs["w4r"], f32).reshape(128, 128)
    w4i = np.asarray(inputs["w4i"], f32).reshape(128, 128)
    W4 = np.zeros((128, 4 * 128), f32)
    W4[:, 0:128] = w4r.T
    W4[:, 128:256] = -w4i.T
    W4[:, 256:384] = w4i.T
    W4[:, 384:512] = w4r.T
    B4 = np.stack([np.asarray(inputs["b4r"], f32),
                   np.asarray(inputs["b4i"], f32)], axis=1)

    w5r = np.asarray(inputs["w5r"], f32).reshape(64, 128)
    w5i = np.asarray(inputs["w5i"], f32).reshape(64, 128)
    W5 = np.zeros((128, 2 * 128), f32)
    W5[:, 0:64] = w5r.T
    W5[:, 64:128] = w5i.T
    W5[:, 128:192] = -w5i.T
    W5[:, 192:256] = w5r.T
    B5 = np.concatenate([np.asarray(inputs["b5r"], f32),
                         np.asarray(inputs["b5i"], f32)])[:, None]

    return {
        "w1": W1.astype(BF), "w2a": W2a.astype(BF), "w2c": W2c.astype(BF),
        "w2d": W2d.astype(BF),
        "b2": B2, "w3": W3.astype(BF), "b3": B3, "w4": W4.astype(BF),
        "b4": B4, "w5": W5.astype(BF), "b5": B5,
    }


def _prep_x1(xr_b, xi_b, h):
    """Conv1 input slab for one (batch, half): [64, R1, 9, 9, 20] bf16."""
    S = np.zeros((64, R1, 9, 9, 20), np.float32)
    glo = max(0, 4 * h - 1)
    ghi = min(8, 4 * h + 5)
    rlo = glo - (4 * h - 1)
    rhi = ghi - (4 * h - 1) + 1
    for t, (j1, j2, j3) in enumerate(itertools.product(range(3), repeat=3)):
        subr = xr_b[j1:j1 + 17:2, j2:j2 + 17:2, j3:j3 + 17:2, :]
        subi = xi_b[j1:j1 + 17:2, j2:j2 + 17:2, j3:j3 + 17:2, :]
        S[t, rlo:rhi] = subr[glo:ghi + 1]
        S[27 + t, rlo:rhi] = subi[glo:ghi + 1]
    S[54, rlo:rhi] = 1.0
    return S.reshape(64, S1N).astype(BF)


def _prep_fcw(fcw, h):
    out = np.zeros((128, N5), np.float32)
    f = np.asarray(fcw, np.float32).reshape(-1)
    for rr in range(R5):
        g5 = rr + 2 * h
        if h == 1 and rr == 0:
            continue  # overlap row masked on half 1
        out[:, rr * 125:(rr + 1) * 125] = f[g5 * 125:(g5 + 1) * 125][None, :]
    return out


def _make_in_maps(inputs):
    wmaps = _prep_weights(inputs)
    xr = np.asarray(inputs["xr"], np.float32)
    xi = np.asarray(inputs["xi"], np.float32)
    fcw = inputs["fcw"]
    in_maps = []
    for core in range(8):
        b, h = core // 2, core % 2
        m = dict(wmaps)
        m["x1"] = _prep_x1(xr[b, 0], xi[b, 0], h)
        m["fcw"] = _prep_fcw(fcw, h)
        in_maps.append(m)
    return in_maps


def kernel(**inputs):
    if "nc" not in _CACHE:
        _CACHE["nc"] = _build_nc()
    nc = _CACHE["nc"]

    in_maps = _make_in_maps(inputs)

    res = run_bass_kernel_spmd(nc, in_maps, core_ids=list(range(8)))

    fcb = np.asarray(inputs["fcb"], np.float32)
    yr = np.zeros((NB, 64, 1), np.float32)
    yi = np.zeros((NB, 64, 1), np.float32)
    for b in range(NB):
        p0 = res.results[2 * b]["out"]
        p1 = res.results[2 * b + 1]["out"]
        s = p0 + p1
        yr[b] = s[0:64] + fcb[0]
        yi[b] = s[64:128]
    return np.stack([yr, yi]).astype(np.float32)


# revision 28
# speedup vs baseline: 1.0988x; 1.0616x over previous
"""Trainium2 Bass kernel for nn_LASLNNet (complex-valued 4D CNN).

Strategy (8 NeuronCores, SPMD single program):
  - core c handles (batch b = c//2, spatial half h = c%2) -> 4 x 2 split.
  - All complex convs are computed as real matmuls with doubled channels:
      [yr; yi] = [[Wr, Wi], [-Wi, Wr]]^T @ [xr; xi]
  - Every matmul keeps K uniform (the PE array reconfigures between
    different contraction sizes at a ~2.3x throughput penalty, so K=64
    slots are padded / packed rather than issued as narrow matmuls).
  - conv1 (k=3,s=2): im2col slabs on host (27 (j1,j2,j3) tap slabs,
    K=55 incl. a bias/ones row); j4 handled as 3 PSUM-accumulated
    K=64 matmuls with step-2 rhs reads. M=128: output channels are
    duplicated in the weight columns so PSUM partitions 64..127 carry a
    second copy used to build shifted x2 replicas without DMA.
  - x2 store: d4-padded flat grid [block(d1) 7, d2 9, d3 9, d4 10];
    x2t partitions 0..63 hold x2, partitions 64..127 hold x2 shifted +1
    (written by a second activation with dst offset -1, pad columns
    supplied by the initial memset). A second tile x2s810 holds x2 on
    partitions 0..63 (DVE block copies) and x2 shifted +810 = one d1
    block on partitions 64..127 (third activation per conv1 row).
  - conv2 (k=3,s=1,p=1): per (row, o2-group) PSUM accumulation of
    45 K=128 matmuls: 27 (j1,j2,j3) taps with j4 in {0,1} fused via the
    +1 replica; 9 (j2,j3) taps with j4=2, j1 in {0,1} fused via the
    +810 replica; 9 (j2,j3) taps with j4=2, j1=2 with zero upper-half
    weights. Edge taps restrict (o2,o3) ranges via strided APs; PSUM
    has_written semantics make partial-region accumulation correct
    (the first matmul of each group is the full-region interior tap).
  - conv3/4/5 (1x1): plain matmuls on a compact layout.
  - FC: on-chip mul+reduce against host-sliced fcw; final cross-half
    sum + fc bias on host (each core returns a [128,1] partial).
  - dtype: bf16 matmul operands, fp32 PSUM/copies.

Spatial split along first output spatial dim D1 (9 rows):
  half 0 -> conv2..4 rows 0..4, half 1 -> rows 4..8 (row 4 duplicated);
  conv5 rows {0,1,2} / {2,3,4} (row 2 duplicated, masked via zeroed fcw).
"""

import itertools

import numpy as np
import ml_dtypes

import concourse.bacc as bacc
import concourse.mybir as mybir
from concourse.tile import TileContext
from concourse.bass_utils import run_bass_kernel_spmd

F32 = mybir.dt.float32
BF16 = mybir.dt.bfloat16
BF = ml_dtypes.bfloat16

NB = 4            # batch
R1 = 7            # conv1 rows computed per core (incl. dummy edge rows)
R2 = 5            # conv2/3/4 rows per core
R5 = 3            # conv5 rows per core
D4P = 10          # d4-padded inner dim (9 valid + 1 zero)
BLK = 9 * 9 * D4P                # 810, one d1-block of x2
X2N = R1 * BLK                   # logical x2 elements per partition
S1R = 9 * 9 * 20                 # 1620, conv1 slab elements per row
S1N = R1 * S1R                   # 11340 conv1 slab elements per partition
N3 = R2 * 729                    # 3645 compact columns for conv3/4
N5 = R5 * 125                    # 375 conv5 output columns

_CACHE = {}


def _build_nc():
    nc = bacc.Bacc("TRN2", target_bir_lowering=False, debug=False)

    x1_d = nc.dram_tensor("x1", [64, S1N], BF16, kind="ExternalInput")
    w1_d = nc.dram_tensor("w1", [64, 3 * 128], BF16, kind="ExternalInput")
    w2a_d = nc.dram_tensor("w2a", [128, 27 * 128], BF16, kind="ExternalInput")
    w2c_d = nc.dram_tensor("w2c", [128, 9 * 128], BF16, kind="ExternalInput")
    w2d_d = nc.dram_tensor("w2d", [128, 9 * 128], BF16, kind="ExternalInput")
    b2_d = nc.dram_tensor("b2", [128, 1], F32, kind="ExternalInput")
    w3_d = nc.dram_tensor("w3", [128, 2 * 128], BF16, kind="ExternalInput")
    b3_d = nc.dram_tensor("b3", [128, 2], F32, kind="ExternalInput")
    w4_d = nc.dram_tensor("w4", [128, 4 * 128], BF16, kind="ExternalInput")
    b4_d = nc.dram_tensor("b4", [128, 2], F32, kind="ExternalInput")
    w5_d = nc.dram_tensor("w5", [128, 2 * 128], BF16, kind="ExternalInput")
    b5_d = nc.dram_tensor("b5", [128, 1], F32, kind="ExternalInput")
    fcw_d = nc.dram_tensor("fcw", [128, N5], F32, kind="ExternalInput")
    out_d = nc.dram_tensor("out", [128, 1], F32, kind="ExternalOutput")

    Relu = mybir.ActivationFunctionType.Relu

    with TileContext(nc) as tc:
        with tc.tile_pool(name="sb", bufs=1) as pool, \
             tc.tile_pool(name="ps", bufs=8, space="PSUM") as pp:
            x1t = pool.tile([64, S1N], BF16, tag="x1")
            w1t = pool.tile([64, 3 * 128], BF16, tag="w1")
            # x2 store: [1 lead margin][R1 blocks of BLK][1 tail margin]
            x2t = pool.tile([128, X2N + 92], BF16, tag="x2")
            x2s8 = pool.tile([128, X2N + 92], BF16, tag="x2s8")
            w2at = pool.tile([128, 27 * 128], BF16, tag="w2a")
            w2ct = pool.tile([128, 9 * 128], BF16, tag="w2c")
            w2dt = pool.tile([128, 9 * 128], BF16, tag="w2d")
            b2t = pool.tile([128, 1], F32, tag="b2")
            x3t = pool.tile([128, N3], BF16, tag="x3")
            w3t = pool.tile([128, 2 * 128], BF16, tag="w3")
            b3t = pool.tile([128, 2], F32, tag="b3")
            x4t = pool.tile([128, 2 * N3], BF16, tag="x4")
            x4bt = pool.tile([128, 2 * N3], BF16, tag="x4b")
            w4t = pool.tile([128, 4 * 128], BF16, tag="w4")
            b4t = pool.tile([128, 2], F32, tag="b4")
            w5t = pool.tile([128, 2 * 128], BF16, tag="w5")
            b5t = pool.tile([128, 1], F32, tag="b5")
            x5t = pool.tile([128, N5], F32, tag="x5")
            fcwt = pool.tile([128, N5], F32, tag="fcw")
            prodt = pool.tile([128, N5], F32, tag="prod")
            fct = pool.tile([128, 1], F32, tag="fc")

            # weights first (small w1 unblocks conv1), then x1 row chunks
            # so conv1 row r only waits for its own slab chunk.
            nc.sync.dma_start(w1t[:, :], w1_d[:, :])
            for (ra, rb) in ((0, 2), (2, 4), (4, 7)):
                nc.sync.dma_start(x1t[:, ra * S1R:rb * S1R],
                                  x1_d[:, ra * S1R:rb * S1R])
            nc.sync.dma_start(w2at[:, :], w2a_d[:, :])
            nc.sync.dma_start(w2ct[:, :], w2c_d[:, :])
            nc.sync.dma_start(w2dt[:, :], w2d_d[:, :])
            nc.sync.dma_start(b2t[:, :], b2_d[:, :])
            nc.sync.dma_start(w3t[:, :], w3_d[:, :])
            nc.sync.dma_start(b3t[:, :], b3_d[:, :])
            nc.sync.dma_start(w4t[:, :], w4_d[:, :])
            nc.sync.dma_start(b4t[:, :], b4_d[:, :])
            nc.sync.dma_start(w5t[:, :], w5_d[:, :])
            nc.sync.dma_start(b5t[:, :], b5_d[:, :])
            nc.sync.dma_start(fcwt[:, :], fcw_d[:, :])

            # ---------------- conv1 ----------------
            # slab view: [r(R1), o2(9), o3(9), d4(20)]
            s1v = x1t.rearrange("p (r a b c) -> p r a b c", r=R1, a=9, b=9, c=20)
            # x2 logical views. Lower (alloc offset 1): plain x2.
            x2v = x2t[:, 1:1 + X2N].rearrange(
                "p (r a b c) -> p r a b c", r=R1, a=9, b=9, c=D4P)
            # Upper of x2t (alloc offset 0): x2 shifted +1; the d4 pad
            # column of each cell is never written -> stays 0 from the
            # memset, which is exactly value x2[pad]=0 shifted into place.
            x2u = x2t[:, 0:X2N].rearrange(
                "p (r a b c) -> p r a b c", r=R1, a=9, b=9, c=D4P)
            # Upper of x2s8 (alloc offset 1): x2 shifted +810 (one block):
            # conv1 row r output is written at block slot r-1.
            x2s8u = x2s8[:, 1:1 + X2N].rearrange(
                "p (r a b c) -> p r a b c", r=R1, a=9, b=9, c=D4P)

            # Zero the grids (pad columns, margins, unwritten gaps). Must
            # cover full cells, not just the k=9 pads: a pads-only memset
            # is region-disjoint from the activations, so nothing would
            # order it before them, and k=8/k=9 share a 4-byte word ->
            # engine write race. Overlapping regions serialize via Tile
            # deps; gpsimd runs these during the x1 DMA, off the critical
            # path. x2s8's lower half needs no zeroing (DVE block copies
            # are its only writer and cover every read).
            nc.gpsimd.memset(x2t[0:64, 0:X2N + 1], 0)
            nc.vector.memset(x2t[64:128, 0:X2N], 0)
            nc.vector.memset(x2s8[64:128, 1:1 + X2N], 0)
            # conv5 row 2 reads the (never-computed) o2 tail of x4b row 4;
            # zero it (1-col overlap with the conv4 act region orders the
            # memset before the act, avoiding a shared-word write race).
            for mh in range(2):
                t0c = mh * N3 + 4 * 729 + 404
                nc.gpsimd.memset(x4bt[:, t0c:(mh + 1) * N3], 0)

            for r in range(R1):
                for (o2s, c2g) in ((0, 5), (5, 4)):
                    n = c2g * 81
                    ps1 = pp.tile([128, 512], F32, tag="ps")
                    ps1v = ps1[:, :n].rearrange("p (a b c) -> p a b c",
                                                a=c2g, b=9, c=9)
                    for j4 in range(3):
                        rhs = s1v[:, r, o2s:o2s + c2g, :, j4:j4 + 17:2]
                        nc.tensor.matmul(
                            ps1v[:, :, :, :],
                            w1t[:, j4 * 128:(j4 + 1) * 128],
                            rhs,
                            start=(j4 == 0), stop=(j4 == 2))
                    nc.scalar.activation(
                        x2v[0:64, r, o2s:o2s + c2g, :, 0:9],
                        ps1v[0:64, :, :, :],
                        Relu)
                    nc.vector.tensor_relu(
                        x2u[64:128, r, o2s:o2s + c2g, :, 0:9],
                        ps1v[64:128, :, :, :])
                    if r >= 1:
                        nc.scalar.activation(
                            x2s8u[64:128, r - 1, o2s:o2s + c2g, :, 0:9],
                            ps1v[64:128, :, :, :],
                            Relu)
                # lower half of x2s8: plain copy of the finished block
                # (same partitions -> DVE can do it; covers pad columns)
                if r < 5:
                    nc.vector.tensor_copy(
                        x2s8[0:64, 1 + r * BLK:1 + (r + 1) * BLK],
                        x2t[0:64, 1 + r * BLK:1 + (r + 1) * BLK])

            # ---------------- conv2 ----------------
            # taps ordered interior-first so the first matmul of each PSUM
            # group covers the full region (has_written correctness).
            taps = sorted(itertools.product(range(3), repeat=3),
                          key=lambda t: (t != (1, 1, 1)))
            taps9 = list(itertools.product(range(3), repeat=2))
            x3v = x3t.rearrange("p (r a b c) -> p r a b c", r=R2, a=9, b=9, c=9)

            def conv2_row(r):
                # row 4 is split between the halves along o2 (each half in
                # its own mirrored coords computes the (0,5) group only;
                # the o2=4 overlap column is de-duplicated via fcw masks)
                for (o2s, c2g) in (((0, 5), (5, 4)) if r < 4 else ((0, 5),)):
                    n = c2g * 81
                    ps2 = pp.tile([128, 512], F32, tag="ps")
                    ps2v = ps2[:, :n].rearrange("p (a b c) -> p a b c",
                                                a=c2g, b=9, c=9)

                    def region(j2, j3):
                        lo2 = max(o2s, 1 - j2)
                        hi2 = min(o2s + c2g, 10 - j2)
                        lo3 = max(0, 1 - j3)
                        hi3 = min(9, 10 - j3)
                        return lo2, hi2, lo3, hi3

                    # 27 (j1,j2,j3) taps, j4 in {0,1} via the +1 replica
                    for ti, (j1, j2, j3) in enumerate(taps):
                        lo2, hi2, lo3, hi3 = region(j2, j3)
                        c2 = hi2 - lo2
                        c3 = hi3 - lo3
                        out_ap = ps2v[:, lo2 - o2s:hi2 - o2s, lo3:hi3, :]
                        t27 = j1 * 9 + j2 * 3 + j3
                        # alloc base for (o2=lo2, o3=lo3, o4=0), j4=0 on the
                        # base partitions (the +1 alloc offset and the -1
                        # d4 pad shift cancel):
                        base0 = ((r + j1) * BLK + (lo2 + j2 - 1) * 90
                                 + (lo3 + j3 - 1) * D4P)
                        rhs0 = x2t[:, base0:base0 + c2 * 90].rearrange(
                            "p (a b c) -> p a b c", a=c2, b=9, c=D4P)[
                            :, :, 0:c3, 0:9]
                        nc.tensor.matmul(
                            out_ap,
                            w2at[:, t27 * 128:(t27 + 1) * 128],
                            rhs0,
                            start=(ti == 0), stop=False)
                    # 9 (j2,j3) taps, j4=2, j1 in {0,1} via the +810 replica
                    for t9, (j2, j3) in enumerate(taps9):
                        lo2, hi2, lo3, hi3 = region(j2, j3)
                        c2 = hi2 - lo2
                        c3 = hi3 - lo3
                        out_ap = ps2v[:, lo2 - o2s:hi2 - o2s, lo3:hi3, :]
                        base = (r * BLK + (lo2 + j2 - 1) * 90
                                + (lo3 + j3 - 1) * D4P + 2)
                        rhs = x2s8[:, base:base + c2 * 90].rearrange(
                            "p (a b c) -> p a b c", a=c2, b=9, c=D4P)[
                            :, :, 0:c3, 0:9]
                        nc.tensor.matmul(
                            out_ap,
                            w2ct[:, t9 * 128:(t9 + 1) * 128],
                            rhs,
                            start=False, stop=False)
                    # 9 (j2,j3) taps, j4=2, j1=2: zero upper-half weights
                    for t9, (j2, j3) in enumerate(taps9):
                        lo2, hi2, lo3, hi3 = region(j2, j3)
                        c2 = hi2 - lo2
                        c3 = hi3 - lo3
                        out_ap = ps2v[:, lo2 - o2s:hi2 - o2s, lo3:hi3, :]
                        base = ((r + 2) * BLK + (lo2 + j2 - 1) * 90
                                + (lo3 + j3 - 1) * D4P + 2)
                        rhs = x2t[:, base:base + c2 * 90].rearrange(
                            "p (a b c) -> p a b c", a=c2, b=9, c=D4P)[
                            :, :, 0:c3, 0:9]
                        nc.tensor.matmul(
                            out_ap,
                            w2dt[:, t9 * 128:(t9 + 1) * 128],
                            rhs,
                            start=False, stop=(t9 == 8))
                    nc.scalar.activation(
                        x3v[:, r, o2s:o2s + c2g, :, :],
                        ps2v[:, :, :, :],
                        Relu, bias=b2t[:, :])

            # ---------------- conv3/4/5 per-row emitters ----------------
            # Row-aligned chunks so a row's 1x1 convs can ride right
            # behind the producing activations (interleaved below).
            rchunks = ((0, 405), (405, 324))

            def conv3_row(r):
                rch = rchunks if r < 4 else rchunks[:1]
                for (pos, sz) in rch:
                    for mh in range(2):
                        p0 = r * 729 + pos
                        ps3 = pp.tile([128, 512], F32, tag="ps")
                        nc.tensor.matmul(
                            ps3[:, :sz],
                            w3t[:, mh * 128:(mh + 1) * 128],
                            x3t[:, p0:p0 + sz],
                            start=True, stop=True)
                        nc.scalar.activation(
                            x4t[:, mh * N3 + p0:mh * N3 + p0 + sz],
                            ps3[:, :sz],
                            Relu, bias=b3t[:, mh:mh + 1])

            def conv4_row(r):
                rch = rchunks if r < 4 else rchunks[:1]
                for (pos, sz) in rch:
                    for mh in range(2):
                        p0 = r * 729 + pos
                        ps4 = pp.tile([128, 512], F32, tag="ps")
                        nc.tensor.matmul(
                            ps4[:, :sz],
                            w4t[:, (mh * 2) * 128:(mh * 2 + 1) * 128],
                            x4t[:, p0:p0 + sz],
                            start=True, stop=False)
                        nc.tensor.matmul(
                            ps4[:, :sz],
                            w4t[:, (mh * 2 + 1) * 128:(mh * 2 + 2) * 128],
                            x4t[:, N3 + p0:N3 + p0 + sz],
                            start=False, stop=True)
                        nc.scalar.activation(
                            x4bt[:, mh * N3 + p0:mh * N3 + p0 + sz],
                            ps4[:, :sz],
                            Relu, bias=b4t[:, mh:mh + 1])

            # x4b view: [mb(2), r(R2), o2(9), o3(9), o4(9)]
            x4bv = x4bt.rearrange("p (m r a b c) -> p m r a b c",
                                  m=2, r=R2, a=9, b=9, c=9)

            def conv5_row(rr):
                ps5 = pp.tile([128, 512], F32, tag="ps")
                for mb in range(2):
                    rhs = x4bv[:, mb, 2 * rr, 0:9:2, 0:9:2, 0:9:2]
                    nc.tensor.matmul(
                        ps5[:, :125],
                        w5t[:, mb * 128:(mb + 1) * 128],
                        rhs,
                        start=(mb == 0), stop=(mb == 1))
                nc.scalar.activation(
                    x5t[:, rr * 125:(rr + 1) * 125],
                    ps5[:, :125],
                    Relu, bias=b5t[:, :])

            # Interleaved schedule: downstream layers lag far enough that
            # their input activations are already drained when the PE
            # reaches them, so the tensor engine never stalls.
            for r in range(R2):
                conv2_row(r)
                if r >= 1:
                    conv3_row(r - 1)
                if r >= 2:
                    conv4_row(r - 2)
                if r == 4:
                    conv5_row(0)
            conv3_row(4)
            conv4_row(3)
            conv5_row(1)
            conv4_row(4)
            conv5_row(2)

            # ---------------- FC partials ----------------
            nc.vector.tensor_mul(prodt[:, :], x5t[:, :], fcwt[:, :])
            nc.vector.reduce_sum(fct[:, :], prodt[:, :],
                                 axis=mybir.AxisListType.X)

            nc.sync.dma_start(out_d[:, :], fct[:, :])

    nc.compile()
    return nc


# ---------------- host-side data prep ----------------

def _cplx_block(wr_t, wi_t):
    """[32ci r; 32ci i] x [64co r | 64co i] real-matmul block."""
    f32 = np.float32
    B = np.zeros((64, 128), f32)
    B[0:32, 0:64] = wr_t
    B[0:32, 64:128] = wi_t
    B[32:64, 0:64] = -wi_t
    B[32:64, 64:128] = wr_t
    return B


def _prep_conv12_weights(inputs, h):
    """w1/w2a/w2c/w2d for one half. h=1 cores see d1/d2-mirrored inputs,
    so their conv taps are flipped along j1 and j2."""
    f32 = np.float32
    fl = slice(None) if h == 0 else slice(None, None, -1)
    w1r = np.asarray(inputs["w1r"], f32)[:, 0][:, fl, fl]   # [32, 3,3,3,3]
    w1i = np.asarray(inputs["w1i"], f32)[:, 0][:, fl, fl]
    # [t27, j4, co]
    w1r_t = w1r.transpose(1, 2, 3, 4, 0).reshape(27, 3, 32)
    w1i_t = w1i.transpose(1, 2, 3, 4, 0).reshape(27, 3, 32)
    # [64, 3*128]: per j4 block, M=128 with duplicated 64-wide halves so
    # PSUM partitions 64..127 carry a copy (used for shifted replicas).
    W1 = np.zeros((64, 3 * 128), f32)
    for j4 in range(3):
        blk = np.zeros((64, 64), f32)
        blk[0:27, 0:32] = w1r_t[:, j4]
        blk[0:27, 32:64] = w1i_t[:, j4]
        blk[27:54, 0:32] = -w1i_t[:, j4]
        blk[27:54, 32:64] = w1r_t[:, j4]
        if j4 == 0:
            blk[54, 0:32] = np.asarray(inputs["b1r"], f32)
            blk[54, 32:64] = np.asarray(inputs["b1i"], f32)
        W1[:, j4 * 128:j4 * 128 + 64] = blk
        W1[:, j4 * 128 + 64:(j4 + 1) * 128] = blk

    w2r = np.asarray(inputs["w2r"], f32)[:, :, fl, fl]   # [64, 32, 3,3,3,3]
    w2i = np.asarray(inputs["w2i"], f32)[:, :, fl, fl]
    # [j1, j2, j3, j4, ci, co]
    w2r_t = w2r.transpose(2, 3, 4, 5, 1, 0)
    w2i_t = w2i.transpose(2, 3, 4, 5, 1, 0)
    # w2a: 27 (j1,j2,j3) taps, rows 0:64 = j4=0, rows 64:128 = j4=1
    W2a = np.zeros((128, 27 * 128), f32)
    for t, (j1, j2, j3) in enumerate(itertools.product(range(3), repeat=3)):
        W2a[0:64, t * 128:(t + 1) * 128] = _cplx_block(
            w2r_t[j1, j2, j3, 0], w2i_t[j1, j2, j3, 0])
        W2a[64:128, t * 128:(t + 1) * 128] = _cplx_block(
            w2r_t[j1, j2, j3, 1], w2i_t[j1, j2, j3, 1])
    # w2c: 9 (j2,j3) taps at j4=2, rows 0:64 = j1=0, rows 64:128 = j1=1
    W2c = np.zeros((128, 9 * 128), f32)
    W2d = np.zeros((128, 9 * 128), f32)
    for t, (j2, j3) in enumerate(itertools.product(range(3), repeat=2)):
        W2c[0:64, t * 128:(t + 1) * 128] = _cplx_block(
            w2r_t[0, j2, j3, 2], w2i_t[0, j2, j3, 2])
        W2c[64:128, t * 128:(t + 1) * 128] = _cplx_block(
            w2r_t[1, j2, j3, 2], w2i_t[1, j2, j3, 2])
        W2d[0:64, t * 128:(t + 1) * 128] = _cplx_block(
            w2r_t[2, j2, j3, 2], w2i_t[2, j2, j3, 2])
    return {"w1": W1.astype(BF), "w2a": W2a.astype(BF),
            "w2c": W2c.astype(BF), "w2d": W2d.astype(BF)}


def _prep_weights(inputs):
    f32 = np.float32
    B2 = np.concatenate([np.asarray(inputs["b2r"], f32),
                         np.asarray(inputs["b2i"], f32)])[:, None]

    w3r = np.asarray(inputs["w3r"], f32).reshape(128, 64)
    w3i = np.asarray(inputs["w3i"], f32).reshape(128, 64)
    W3 = np.zeros((128, 2 * 128), f32)
    W3[0:64, 0:128] = w3r.T
    W3[64:128, 0:128] = -w3i.T
    W3[0:64, 128:256] = w3i.T
    W3[64:128, 128:256] = w3r.T
    B3 = np.stack([np.asarray(inputs["b3r"], f32),
                   np.asarray(inputs["b3i"], f32)], axis=1)

    w4r = np.asarray(inputs["w4r"], f32).reshape(128, 128)
    w4i = np.asarray(inputs["w4i"], f32).reshape(128, 128)
    W4 = np.zeros((128, 4 * 128), f32)
    W4[:, 0:128] = w4r.T
    W4[:, 128:256] = -w4i.T
    W4[:, 256:384] = w4i.T
    W4[:, 384:512] = w4r.T
    B4 = np.stack([np.asarray(inputs["b4r"], f32),
                   np.asarray(inputs["b4i"], f32)], axis=1)

    w5r = np.asarray(inputs["w5r"], f32).reshape(64, 128)
    w5i = np.asarray(inputs["w5i"], f32).reshape(64, 128)
    W5 = np.zeros((128, 2 * 128), f32)
    W5[:, 0:64] = w5r.T
    W5[:, 64:128] = w5i.T
    W5[:, 128:192] = -w5i.T
    W5[:, 192:256] = w5r.T
    B5 = np.concatenate([np.asarray(inputs["b5r"], f32),
                         np.asarray(inputs["b5i"], f32)])[:, None]

    return {
        "b2": B2, "w3": W3.astype(BF), "b3": B3, "w4": W4.astype(BF),
        "b4": B4, "w5": W5.astype(BF), "b5": B5,
    }


def _mirror_x(x_b):
    """d1/d2 double mirror of one batch's [20,20,20,20] input: only the
    windows [0,18] feed the convs, so mirror those and leave index 19."""
    xf = np.zeros_like(x_b)
    xf[0:19, 0:19] = x_b[18::-1, 18::-1]
    return xf


def _prep_x1(xr_b, xi_b):
    """Conv1 input slab (half-0 geometry; half 1 passes mirrored x):
    [64, R1, 9, 9, 20] bf16 with a dummy zero row at block 0."""
    S = np.zeros((64, R1, 9, 9, 20), np.float32)
    for t, (j1, j2, j3) in enumerate(itertools.product(range(3), repeat=3)):
        subr = xr_b[j1:j1 + 17:2, j2:j2 + 17:2, j3:j3 + 17:2, :]
        subi = xi_b[j1:j1 + 17:2, j2:j2 + 17:2, j3:j3 + 17:2, :]
        S[t, 1:7] = subr[0:6]
        S[27 + t, 1:7] = subi[0:6]
    S[54, 1:7] = 1.0
    return S.reshape(64, S1N).astype(BF)


def _prep_fcw(fcw, h):
    """Per-core fcw slice in local x5 column order. Local col
    (rr, i2, i3, i4) maps to global conv5 output (o1h, o2h, o3h, o4h) =
    (rr, i2, i3, i4) for h=0 and (4-rr, 4-i2, i3, i4) for h=1 (d1/d2
    mirror). Masks: h=0 owns the (o1h=2, o2h=2) overlap column; each
    half's row-4 o2 tail (i2 beyond its split) is garbage/foreign."""
    out = np.zeros((128, N5), np.float32)
    f = np.asarray(fcw, np.float32).reshape(5, 5, 5, 5)
    for rr in range(R5):
        for i2 in range(5):
            if rr == 2 and i2 >= (3 if h == 0 else 2):
                continue
            g = f[rr, i2] if h == 0 else f[4 - rr, 4 - i2]
            cols = rr * 125 + i2 * 25 + np.arange(25)
            out[:, cols] = g.reshape(-1)[None, :]
    return out


def _make_in_maps(inputs):
    shared = _prep_weights(inputs)
    whalf = [_prep_conv12_weights(inputs, h) for h in range(2)]
    xr = np.asarray(inputs["xr"], np.float32)
    xi = np.asarray(inputs["xi"], np.float32)
    fcw = inputs["fcw"]
    fcwh = [_prep_fcw(fcw, h) for h in range(2)]
    in_maps = []
    for core in range(8):
        b, h = core // 2, core % 2
        m = dict(shared)
        m.update(whalf[h])
        xr_b, xi_b = xr[b, 0], xi[b, 0]
        if h == 1:
            xr_b, xi_b = _mirror_x(xr_b), _mirror_x(xi_b)
        m["x1"] = _prep_x1(xr_b, xi_b)
        m["fcw"] = fcwh[h]
        in_maps.append(m)
    return in_maps


def kernel(**inputs):
    if "nc" not in _CACHE:
        _CACHE["nc"] = _build_nc()
    nc = _CACHE["nc"]

    in_maps = _make_in_maps(inputs)

    res = run_bass_kernel_spmd(nc, in_maps, core_ids=list(range(8)))

    fcb = np.asarray(inputs["fcb"], np.float32)
    yr = np.zeros((NB, 64, 1), np.float32)
    yi = np.zeros((NB, 64, 1), np.float32)
    for b in range(NB):
        p0 = res.results[2 * b]["out"]
        p1 = res.results[2 * b + 1]["out"]
        s = p0 + p1
        yr[b] = s[0:64] + fcb[0]
        yi[b] = s[64:128]
    return np.stack([yr, yi]).astype(np.float32)
